# revision 11
# baseline (speedup 1.0000x reference)
"""Trainium2 Bass kernel for nn_MixtureOfExperts (argmax-routed SwiGLU MoE).

Strategy (expert-parallel across 8 NeuronCores):
  - Host computes router logits (fp64 matmul, tiny: 4096x1024x8) and the
    argmax expert per token.  Top-2 logit gaps are >=1.7e-4 while fp32
    rounding noise is ~1e-6, so routing is insensitive to arithmetic order.
  - Tokens are grouped by selected expert; each core receives only its
    expert's tokens (padded to a common capacity C) plus that expert's
    gate/up/down banks.  Each core computes the SwiGLU for its tokens only:
        h = silu(x @ gw) * (x @ uw);  y = h @ dw
    This does 1/E of the reference FLOPs (the reference computes all E
    experts densely and discards all but the argmax one).
  - Host scatters per-core outputs back to token positions.

Layout: x ships pre-transposed/k-blocked as [128, KD*C] so the contraction
dim D lands on SBUF partitions.  mm1 produces h^T tiles [128h, C] which are
directly the *moving* operand for mm2 computed in transposed orientation:
    y^T[D, C] = dw^T-contracted-over-H; lhsT = dw k-slices [128h, 128d]
so the mm2 free dim is the token count (no partition-padding waste on the
ragged last token tile) and the host transposes y^T back (cheap numpy).

Matmul dtype: bfloat16.  The PE streams bf16 at the same 1 cycle/row as
float32r, but LDWEIGHTS (the stationary-operand load serialized on the
Tensor queue) gets Fast Weight Load only for sub-4-byte dtypes: fp32r
weight tiles load at ~188ns and the 672 loads dominated the fp32r
kernel's critical path.  bf16 also halves DMA.  End-to-end bf16 error vs
the fp32 reference is ~4e-3, inside the 2e-2 gate.

All weights stay SBUF-resident (gu 8.4MB + dw 4.2MB + x/h ~3.5MB < 24MB),
packed host-side so every weight load is one contiguous DMA per H-chunk:
DMA *issue* slots (~0.6us each on an engine queue) were the limiter on
arming the pipeline, not HBM bandwidth.
"""

import ml_dtypes
import numpy as np

import concourse.mybir as mybir
import concourse.tile as tile
from concourse import bacc
from concourse.bass_utils import run_bass_kernel_spmd

B, T, D, E, H = 4, 1024, 1024, 8, 2048
BT = B * T
NCORES = 8
P = 128
KD = D // P   # k-tiles for mm1 (contraction over D)
KH = H // P   # k-tiles for mm2 (contraction over H)
ND = D // P   # output d-tiles for mm2 (partition dim of y^T)
F32 = mybir.dt.float32
BF16 = mybir.dt.bfloat16
BF16_NP = ml_dtypes.bfloat16

# "fp32" : exact fp32 matmuls (4 cycles/row on PE)
# "bf16" : bf16 operands (1 cycle/row, fast weight loads, half DMA)
MM_MODE = "bf16"

# gate/up weight chunks over H.  First chunks are small so the critical
# prefix (x + first weight chunk) is minimal before the PE can start.
H_CHUNKS = [(0, 128), (128, 128), (256, 256), (512, 512), (1024, 512),
            (1536, 512)]
assert sum(c for _, c in H_CHUNKS) == H

_BUILD_CACHE = {}


def _balanced_chunks(total, step):
    """Split `total` into <=step chunks, as equal as possible (32-aligned)."""
    n = -(-total // step)
    base = -(-total // (n * 32)) * 32
    out = []
    o = 0
    while o < total:
        sz = min(base, total - o)
        out.append((o, sz))
        o += sz
    return out


def _build(C, mm_mode):
    """Build the per-core SPMD Bass kernel for token capacity C."""
    n_chunks = _balanced_chunks(C, 512)   # token tiles in the free dim

    mdt = BF16 if mm_mode == "bf16" else F32

    nc = bacc.Bacc("TRN2", target_bir_lowering=False, debug=False)
    # xt packed partition-major: [128, KD*C], block k = x^T[k*128:(k+1)*128,:]
    xt = nc.dram_tensor("xt", [P, KD * C], mdt, kind="ExternalInput")
    # gu packed per H-chunk, k-blocked: chunk ci occupies KD*2*hcn columns,
    # laid out [k][gate hcn | up hcn]
    gu = nc.dram_tensor("gu", [P, KD * 2 * H], mdt, kind="ExternalInput")
    # dw packed k-blocked: [128, KH*D], block k = dw[k*128:(k+1)*128, :]
    dw = nc.dram_tensor("dw", [P, KH * D], mdt, kind="ExternalInput")
    # transposed output y^T
    yt = nc.dram_tensor("yt", [D, C], F32, kind="ExternalOutput")

    with tile.TileContext(nc) as tc:
        with (
            tc.tile_pool(name="xp", bufs=2) as xp,
            tc.tile_pool(name="hp", bufs=KH) as hp,
            tc.tile_pool(name="w1", bufs=len(H_CHUNKS)) as w1,
            tc.tile_pool(name="w2", bufs=1) as w2,
            tc.tile_pool(name="outp", bufs=4) as outp,
            tc.tile_pool(name="ps", bufs=8, space="PSUM") as ps,
        ):
            # DMA issue costs ~0.6us of serialized queue time per dma_start.
            # Spread issues across engine queues; at the head every engine is
            # idle, so the critical prefix arms in ~1 issue slot.
            head_engines = [nc.sync, nc.scalar, nc.gpsimd]
            body_engines = [nc.sync, nc.scalar]
            _eng_i = [0]

            def dma(engs, dst, src):
                engs[_eng_i[0] % len(engs)].dma_start(dst, src)
                _eng_i[0] += 1

            # ---- input loads ----
            # x: one resident tile per token chunk, layout [128, KD*nn]
            # (k-block k at columns [k*nn:(k+1)*nn]).
            xt3 = xt.rearrange("p (k c) -> p k c", k=KD)
            x_t = {}

            def load_x_chunk(ni, engs, split):
                n0, nn_ = n_chunks[ni]
                t = xp.tile([P, KD * nn_], mdt, tag="x", name=f"x{ni}")
                t3 = t[:].rearrange("p (k c) -> p k c", k=KD)
                step = KD // split
                for j in range(split):
                    ks = slice(j * step, (j + 1) * step)
                    dma(engs, t3[:, ks, :], xt3[:, ks, n0:n0 + nn_])
                x_t[ni] = t

            def x_slice(k, ni):
                nn_ = n_chunks[ni][1]
                return x_t[ni][:, k * nn_:(k + 1) * nn_]

            # gate/up: one resident tile per H-chunk, [128, KD*2*hcn]
            w_t = []
            gu_col = 0
            for ci, (hc0, hcn) in enumerate(H_CHUNKS):
                cols = KD * 2 * hcn
                t = w1.tile([P, cols], mdt, tag="w", name=f"w{ci}")
                split = 2 if ci == 0 else 1
                step = cols // split
                for j in range(split):
                    cs = slice(j * step, (j + 1) * step)
                    dma(head_engines if ci <= 1 else body_engines,
                        t[:, cs], gu[:, gu_col + j * step:gu_col + (j + 1) * step])
                w_t.append(t)
                gu_col += cols
                if ci == 0:
                    # x chunk 0 split in half so the first matmul arms early
                    load_x_chunk(0, head_engines, 2)
                elif ci == 1:
                    for ni in range(1, len(n_chunks)):
                        load_x_chunk(ni, head_engines, 1)

            def w_slice(ci, k, which, hs):
                # which: 0 = gate, 1 = up
                hcn = H_CHUNKS[ci][1]
                off = k * 2 * hcn + which * hcn + hs * P
                return w_t[ci][:, off:off + P]

            # down-proj: one resident tile [128, KH*D], 4-way split DMA
            dwt = w2.tile([P, KH * D], mdt, tag="dw", name="dwt")
            for j in range(4):
                step = KH * D // 4
                dma(head_engines, dwt[:, j * step:(j + 1) * step],
                    dw[:, j * step:(j + 1) * step])

            def dw_slice(k, dt):
                return dwt[:, k * D + dt * P:k * D + dt * P + P]

            # h^T tiles [128, C] (bf16), one per H k-slice
            h_t = [hp.tile([P, C], mdt, tag="h", name=f"h{k}") for k in range(KH)]

            def mm1(ni):
                n0, nn_ = n_chunks[ni]
                for ci, (hc0, hcn) in enumerate(H_CHUNKS):
                    for hs in range(hcn // P):
                        ht = h_t[(hc0 + hs * P) // P]
                        pa = ps.tile([P, 512], F32, tag="ps", name="pa")[:, :nn_]
                        pu = ps.tile([P, 512], F32, tag="ps", name="pu")[:, :nn_]
                        # interleave the gate/up accumulation chains: back-to-
                        # back accumulates into one PSUM bank stall the PE
                        for k in range(KD):
                            nc.tensor.matmul(
                                pa[:, :], lhsT=w_slice(ci, k, 0, hs),
                                rhs=x_slice(k, ni),
                                start=(k == 0), stop=(k == KD - 1),
                            )
                            nc.tensor.matmul(
                                pu[:, :], lhsT=w_slice(ci, k, 1, hs),
                                rhs=x_slice(k, ni),
                                start=(k == 0), stop=(k == KD - 1),
                            )
                        nc.scalar.activation(
                            ht[:, n0:n0 + nn_], pa[:, :],
                            mybir.ActivationFunctionType.Silu,
                        )
                        nc.vector.tensor_mul(
                            ht[:, n0:n0 + nn_], ht[:, n0:n0 + nn_], pu[:, :]
                        )

            def mm2(ni):
                n0, nn_ = n_chunks[ni]
                for dt in range(ND):
                    py = ps.tile([P, 512], F32, tag="ps", name="py")[:, :nn_]
                    for k in range(KH):
                        nc.tensor.matmul(
                            py[:, :], lhsT=dw_slice(k, dt),
                            rhs=h_t[k][:, n0:n0 + nn_],
                            start=(k == 0), stop=(k == KH - 1),
                        )
                    ot = outp.tile([P, 512], F32, tag="out", name="ot")[:, :nn_]
                    nc.vector.tensor_copy(ot[:, :], py[:, :])
                    nc.sync.dma_start(yt[dt * P:(dt + 1) * P, n0:n0 + nn_],
                                      ot[:, :])

            # mm2(chunk i) runs while later chunks' mm1 fills the queue, so
            # output DMA drains throughout instead of in one tail burst.
            for ni in range(len(n_chunks)):
                mm1(ni)
                mm2(ni)

    nc.compile()
    return nc


def _get_kernel(C, mm_mode=None):
    """Build (cached).  Falls back to exact fp32 if the bf16 build fails."""
    mm_mode = mm_mode or MM_MODE
    key = (C, mm_mode)
    if key not in _BUILD_CACHE:
        try:
            _BUILD_CACHE[key] = (_build(C, mm_mode), mm_mode)
        except Exception:
            if mm_mode == "fp32":
                raise
            _BUILD_CACHE[key] = (_build(C, "fp32"), "fp32")
    return _BUILD_CACHE[key]


def _route(xf, gate_w):
    """argmax expert per token, computed in fp64 on host (negligible work)."""
    logits = xf.astype(np.float64) @ np.asarray(gate_w, np.float64).T
    return logits.argmax(axis=1)


def _pack_gu(gw_e, uw_e, ndt):
    """[128, KD*2H]: per H-chunk, k-blocked [k][gate hcn | up hcn]."""
    g3 = gw_e.reshape(KD, P, H)
    u3 = uw_e.reshape(KD, P, H)
    parts = []
    for hc0, hcn in H_CHUNKS:
        blk = np.concatenate(
            [g3[:, :, hc0:hc0 + hcn], u3[:, :, hc0:hc0 + hcn]], axis=2)
        parts.append(blk.transpose(1, 0, 2).reshape(P, KD * 2 * hcn))
    return np.ascontiguousarray(np.concatenate(parts, axis=1)).astype(ndt)


def _pack_dw(dw_e, ndt):
    """[128, KH*D]: k-blocked partition-major."""
    return np.ascontiguousarray(
        dw_e.reshape(KH, P, D).transpose(1, 0, 2).reshape(P, KH * D)
    ).astype(ndt)


def kernel(x, gate_w, gate_bank, up_bank, down_bank):
    x = np.asarray(x, np.float32)
    assert x.shape == (B, T, D)

    xf = np.ascontiguousarray(x.reshape(BT, D))
    sel = _route(xf, gate_w)
    idx = [np.nonzero(sel == e)[0] for e in range(E)]
    maxc = max(len(i) for i in idx)
    C = max(P, -(-maxc // 32) * 32)

    nc, mode = _get_kernel(C)

    ndt = BF16_NP if mode == "bf16" else np.float32
    gate_bank = np.asarray(gate_bank, np.float32)
    up_bank = np.asarray(up_bank, np.float32)
    down_bank = np.asarray(down_bank, np.float32)

    in_maps = []
    for e in range(E):
        xe = np.zeros((D, C), ndt)
        n = len(idx[e])
        if n:
            xe[:, :n] = xf[idx[e]].T.astype(ndt)
        xe = np.ascontiguousarray(
            xe.reshape(KD, P, C).transpose(1, 0, 2).reshape(P, KD * C))
        in_maps.append({
            "xt": xe,
            "gu": _pack_gu(gate_bank[e], up_bank[e], ndt),
            "dw": _pack_dw(down_bank[e], ndt),
        })

    res = run_bass_kernel_spmd(nc, in_maps, core_ids=list(range(NCORES)))

    out = np.empty((BT, D), np.float32)
    for e in range(E):
        n = len(idx[e])
        if n:
            out[idx[e]] = res.results[e]["yt"][:, :n].T
    return out.reshape(B, T, D)


# revision 12
# speedup vs baseline: 1.0882x; 1.0882x over previous
"""Trainium2 Bass kernel for nn_MixtureOfExperts (argmax-routed SwiGLU MoE).

Strategy (expert-parallel across 8 NeuronCores):
  - Host computes router logits (fp64, tiny) and the argmax expert per
    token.  Top-2 logit gaps are >=1.7e-4 while fp32 rounding noise is
    ~1e-6, so routing is insensitive to arithmetic order.
  - Tokens are grouped by selected expert; core e gets expert e's tokens
    (padded to a common capacity C) plus that expert's banks and computes
        h = silu(x @ gw) * (x @ uw);  y = h @ dw
    (1/E of the reference FLOPs - the reference runs all experts densely).
  - Host scatters per-core outputs back to token positions.

Matmul dtype: bfloat16 (same 1 cycle/row PE stream rate as float32r, but
stationary-operand LDWEIGHTS gets Fast Weight Load - fp32r tiles loaded at
~188ns each and 672 of them saturated the Tensor queue - and DMA halves).
End-to-end bf16 error vs the fp32 reference is ~4e-3 (gate: 2e-2).

mm1 computes h^T tiles [128h, C]; mm2 runs in transposed orientation
    y^T[128d, C] += dw[k][128h, 128d]-as-lhsT @ h^T[k]
so its free dim is the token count (no partition-padding on the ragged
last token tile) and the host transposes y^T back (cheap numpy).

DMA: the 8 cores share device HBM (~358 GB/s/core fair share) and the
~16MB/core of traffic is roofline-limited during the load phase, so
arrival ORDER is everything:
  - gu is host-packed hs-major ([hs][gate|up][k][128]) so mm1 consumes
    weight columns in strict prefix order;
  - every load is issued in consumption priority order, 3-way
    column-split across the sync/scalar/gpsimd issue queues, so all
    three DMA queues deliver the current prefix in lockstep;
  - x/gu/dw are packed so every DMA descriptor run is >=1.3KB
    (576B-run x transfers previously crawled at ~40 GB/s);
  - dw (needed only by mm2, >60us in) loads strictly after gu.
All weights stay SBUF-resident (~14MB < 24MB SBUF); mm1 for the second
token chunk reuses them with zero extra traffic.
"""

import ml_dtypes
import numpy as np

import concourse.mybir as mybir
import concourse.tile as tile
from concourse import bacc
from concourse.bass_utils import run_bass_kernel_spmd

B, T, D, E, H = 4, 1024, 1024, 8, 2048
BT = B * T
NCORES = 8
P = 128
KD = D // P   # k-tiles for mm1 (contraction over D)
KH = H // P   # k-tiles for mm2 (contraction over H); also # of h^T tiles
ND = D // P   # output d-tiles for mm2 (partition dim of y^T)
F32 = mybir.dt.float32
BF16 = mybir.dt.bfloat16
BF16_NP = ml_dtypes.bfloat16

# "fp32" : exact fp32 matmuls (4 cycles/row on PE)
# "bf16" : bf16 operands (1 cycle/row, fast weight loads, half DMA)
MM_MODE = "bf16"

# gu load groups in units of hs (one hs = one [128h x (gate|up) x k] block,
# 0.525 MB).  Small leading groups arm the PE quickly.
W_GROUPS = [(0, 1), (1, 2), (2, 4), (4, 8), (8, 12), (12, 16)]
HS_COLS = 2 * KD * P          # SBUF columns per hs block (2048)

_BUILD_CACHE = {}


def _balanced_chunks(total, step):
    """Split `total` into <=step chunks, as equal as possible (32-aligned)."""
    n = -(-total // step)
    base = -(-total // (n * 32)) * 32
    out = []
    o = 0
    while o < total:
        sz = min(base, total - o)
        out.append((o, sz))
        o += sz
    return out


def _build(C, mm_mode):
    """Build the per-core SPMD Bass kernel for token capacity C."""
    n_chunks = _balanced_chunks(C, 512)   # token tiles in the free dim

    mdt = BF16 if mm_mode == "bf16" else F32

    nc = bacc.Bacc("TRN2", target_bir_lowering=False, debug=False)
    # x^T, chunk-blocked then k-blocked: chunk ni at cols KD*n0, inside it
    # k-block k at cols [k*nn:(k+1)*nn] (contiguous per partition).
    xt = nc.dram_tensor("xt", [P, KD * C], mdt, kind="ExternalInput")
    # gate/up packed hs-major: [hs][gate k0..k7 | up k0..k7], 128 cols each
    gu = nc.dram_tensor("gu", [P, 2 * KD * H], mdt, kind="ExternalInput")
    # down-proj k-blocked: block k = dw[k*128:(k+1)*128, :] at cols [k*D:...]
    dw = nc.dram_tensor("dw", [P, KH * D], mdt, kind="ExternalInput")
    # transposed output y^T
    yt = nc.dram_tensor("yt", [D, C], F32, kind="ExternalOutput")

    with tile.TileContext(nc) as tc:
        with (
            tc.tile_pool(name="xp", bufs=2) as xp,
            tc.tile_pool(name="wg", bufs=1) as wg,
            tc.tile_pool(name="w2", bufs=1) as w2,
            tc.tile_pool(name="hp", bufs=KH) as hp,
            tc.tile_pool(name="outp", bufs=4) as outp,
            tc.tile_pool(name="ps", bufs=8, space="PSUM") as ps,
        ):
            dma_engines = [nc.sync, nc.scalar, nc.gpsimd]

            def load3(dst, src, cols):
                """One logical load, column-split 3 ways so all three DMA
                queues advance the consumption prefix in lockstep."""
                b = -(-cols // (3 * 32)) * 32
                o = 0
                for q in range(3):
                    sz = min(b, cols - o)
                    if sz <= 0:
                        break
                    dma_engines[q].dma_start(dst[:, o:o + sz], src[:, o:o + sz])
                    o += sz

            # resident tiles
            x_t = [xp.tile([P, KD * nn], mdt, tag="x", name=f"x{i}")
                   for i, (n0, nn) in enumerate(n_chunks)]
            wgt = wg.tile([P, 2 * KD * H], mdt, tag="w", name="wgt")
            dwt = w2.tile([P, KH * D], mdt, tag="dw", name="dwt")
            h_t = [hp.tile([P, C], mdt, tag="h", name=f"h{k}")
                   for k in range(KH)]

            # ---- loads, in strict consumption-priority order ----
            def load_x(ni):
                n0, nn = n_chunks[ni]
                load3(x_t[ni][:, :], xt[:, KD * n0:KD * (n0 + nn)], KD * nn)

            def load_w_group(gi):
                h0, h1 = W_GROUPS[gi]
                c0, c1 = h0 * HS_COLS, h1 * HS_COLS
                load3(wgt[:, c0:c1], gu[:, c0:c1], c1 - c0)

            load_x(0)
            load_w_group(0)
            load_w_group(1)
            for ni in range(1, len(n_chunks)):
                load_x(ni)
            for gi in range(2, len(W_GROUPS)):
                load_w_group(gi)
            load3(dwt[:, :], dw[:, :], KH * D)

            def w_slice(k, which, hs):
                off = hs * HS_COLS + which * KD * P + k * P
                return wgt[:, off:off + P]

            def dw_slice(k, dt):
                return dwt[:, k * D + dt * P:k * D + dt * P + P]

            def x_slice(k, ni):
                nn = n_chunks[ni][1]
                return x_t[ni][:, k * nn:(k + 1) * nn]

            def mm1(ni):
                n0, nn = n_chunks[ni]
                for hs in range(KH):
                    ht = h_t[hs]
                    pa = ps.tile([P, 512], F32, tag="ps", name="pa")[:, :nn]
                    pu = ps.tile([P, 512], F32, tag="ps", name="pu")[:, :nn]
                    # interleave the gate/up accumulation chains: back-to-
                    # back accumulates into one PSUM bank stall the PE
                    for k in range(KD):
                        nc.tensor.matmul(
                            pa[:, :], lhsT=w_slice(k, 0, hs),
                            rhs=x_slice(k, ni),
                            start=(k == 0), stop=(k == KD - 1),
                        )
                        nc.tensor.matmul(
                            pu[:, :], lhsT=w_slice(k, 1, hs),
                            rhs=x_slice(k, ni),
                            start=(k == 0), stop=(k == KD - 1),
                        )
                    nc.scalar.activation(
                        ht[:, n0:n0 + nn], pa[:, :],
                        mybir.ActivationFunctionType.Silu,
                    )
                    nc.vector.tensor_mul(
                        ht[:, n0:n0 + nn], ht[:, n0:n0 + nn], pu[:, :]
                    )

            def mm2(ni):
                n0, nn = n_chunks[ni]
                for dt in range(ND):
                    py = ps.tile([P, 512], F32, tag="ps", name="py")[:, :nn]
                    for k in range(KH):
                        nc.tensor.matmul(
                            py[:, :], lhsT=dw_slice(k, dt),
                            rhs=h_t[k][:, n0:n0 + nn],
                            start=(k == 0), stop=(k == KH - 1),
                        )
                    ot = outp.tile([P, 512], F32, tag="out", name="ot")[:, :nn]
                    nc.vector.tensor_copy(ot[:, :], py[:, :])
                    dma_engines[dt % 2].dma_start(
                        yt[dt * P:(dt + 1) * P, n0:n0 + nn], ot[:, :])

            # mm2 strictly after all mm1 so gu loads never compete with dw;
            # outputs drain over mm2's ~25us instead of one tail burst.
            for ni in range(len(n_chunks)):
                mm1(ni)
            for ni in range(len(n_chunks)):
                mm2(ni)

    nc.compile()
    return nc


def _get_kernel(C, mm_mode=None):
    """Build (cached).  Falls back to exact fp32 if the bf16 build fails."""
    mm_mode = mm_mode or MM_MODE
    key = (C, mm_mode)
    if key not in _BUILD_CACHE:
        try:
            _BUILD_CACHE[key] = (_build(C, mm_mode), mm_mode)
        except Exception:
            if mm_mode == "fp32":
                raise
            _BUILD_CACHE[key] = (_build(C, "fp32"), "fp32")
    return _BUILD_CACHE[key]


def _route(xf, gate_w):
    """argmax expert per token, computed in fp64 on host (negligible work)."""
    logits = xf.astype(np.float64) @ np.asarray(gate_w, np.float64).T
    return logits.argmax(axis=1)


def _pack_gu(gw_e, uw_e, ndt):
    """[128, 2*KD*H] hs-major: [hs][gate k0..k7 | up k0..k7] x 128 cols."""
    g = gw_e.reshape(KD, P, KH, P).transpose(1, 2, 0, 3)   # [P, hs, k, 128]
    u = uw_e.reshape(KD, P, KH, P).transpose(1, 2, 0, 3)
    return np.ascontiguousarray(
        np.stack([g, u], axis=2).reshape(P, 2 * KD * H)).astype(ndt)


def _pack_dw(dw_e, ndt):
    """[128, KH*D]: k-blocked partition-major."""
    return np.ascontiguousarray(
        dw_e.reshape(KH, P, D).transpose(1, 0, 2).reshape(P, KH * D)
    ).astype(ndt)


def _pack_x(xe, chunks):
    """[128, KD*C]: chunk-blocked, k-blocked inside each chunk."""
    parts = []
    for n0, nn in chunks:
        parts.append(
            xe[:, n0:n0 + nn].reshape(KD, P, nn).transpose(1, 0, 2)
            .reshape(P, KD * nn))
    return np.ascontiguousarray(np.concatenate(parts, axis=1))


def kernel(x, gate_w, gate_bank, up_bank, down_bank):
    x = np.asarray(x, np.float32)
    assert x.shape == (B, T, D)

    xf = np.ascontiguousarray(x.reshape(BT, D))
    sel = _route(xf, gate_w)
    idx = [np.nonzero(sel == e)[0] for e in range(E)]
    maxc = max(len(i) for i in idx)
    C = max(P, -(-maxc // 32) * 32)
    chunks = _balanced_chunks(C, 512)

    nc, mode = _get_kernel(C)

    ndt = BF16_NP if mode == "bf16" else np.float32
    gate_bank = np.asarray(gate_bank, np.float32)
    up_bank = np.asarray(up_bank, np.float32)
    down_bank = np.asarray(down_bank, np.float32)

    in_maps = []
    for e in range(E):
        xe = np.zeros((D, C), ndt)
        n = len(idx[e])
        if n:
            xe[:, :n] = xf[idx[e]].T.astype(ndt)
        in_maps.append({
            "xt": _pack_x(xe, chunks),
            "gu": _pack_gu(gate_bank[e], up_bank[e], ndt),
            "dw": _pack_dw(down_bank[e], ndt),
        })

    res = run_bass_kernel_spmd(nc, in_maps, core_ids=list(range(NCORES)))

    out = np.empty((BT, D), np.float32)
    for e in range(E):
        n = len(idx[e])
        if n:
            out[idx[e]] = res.results[e]["yt"][:, :n].T
    return out.reshape(B, T, D)


# revision 16
# speedup vs baseline: 1.1329x; 1.0410x over previous
"""Trainium2 Bass kernel for nn_MixtureOfExperts (argmax-routed SwiGLU MoE).

Strategy (expert-parallel across 8 NeuronCores):
  - Host computes router logits (fp64, tiny) and the argmax expert per
    token.  Top-2 logit gaps are >=1.7e-4 while fp32 rounding noise is
    ~1e-6, so routing is insensitive to arithmetic order.
  - Tokens are grouped by selected expert; core e gets expert e's tokens
    (padded to a common capacity C) plus that expert's banks and computes
        h = silu(x @ gw) * (x @ uw);  y = h @ dw
    (1/E of the reference FLOPs - the reference runs all experts densely).
  - Host scatters per-core outputs back to token positions.

Matmul dtype: bfloat16 (same 1 cycle/row PE stream rate as float32r, but
stationary-operand LDWEIGHTS gets Fast Weight Load - fp32r tiles loaded at
~188ns each and 672 of them saturated the Tensor queue - and DMA halves).
End-to-end bf16 error vs the fp32 reference is ~4e-3 (gate: 2e-2).

mm1 computes h^T tiles [128h, C]; mm2 runs in transposed orientation
    y^T[128d, C] += dw[k][128h, 128d]-as-lhsT @ h^T[k]
so its free dim is the token count (no partition-padding on the ragged
last token tile) and the host transposes y^T back (cheap numpy).

DMA: the 8 cores share device HBM (~358 GB/s/core fair share) and the
~16MB/core of traffic is roofline-limited during the load phase, so
arrival ORDER is everything:
  - gu is host-packed hs-major ([hs][gate|up][k][128]) so mm1 consumes
    weight columns in strict prefix order;
  - every load is issued in consumption priority order, 3-way
    column-split across the sync/scalar/gpsimd issue queues, so all
    three DMA queues deliver the current prefix in lockstep;
  - x/gu/dw are packed so every DMA descriptor run is >=1.3KB
    (576B-run x transfers previously crawled at ~40 GB/s);
  - dw (needed only by mm2, >60us in) loads strictly after gu.
All weights stay SBUF-resident (~14MB < 24MB SBUF); mm1 for the second
token chunk reuses them with zero extra traffic.
"""

import ml_dtypes
import numpy as np

import concourse.mybir as mybir
import concourse.tile as tile
from concourse import bacc
from concourse.bass_utils import run_bass_kernel_spmd

B, T, D, E, H = 4, 1024, 1024, 8, 2048
BT = B * T
NCORES = 8
P = 128
KD = D // P   # k-tiles for mm1 (contraction over D)
KH = H // P   # k-tiles for mm2 (contraction over H); also # of h^T tiles
ND = D // P   # output d-tiles for mm2 (partition dim of y^T)
F32 = mybir.dt.float32
BF16 = mybir.dt.bfloat16
BF16_NP = ml_dtypes.bfloat16

# "fp32" : exact fp32 matmuls (4 cycles/row on PE)
# "bf16" : bf16 operands (1 cycle/row, fast weight loads, half DMA)
MM_MODE = "bf16"

# gu load groups in units of hs (one hs = one [128h x (gate|up) x k] block,
# 0.525 MB).  Small leading groups arm the PE quickly.
W_GROUPS = [(0, 1), (1, 2), (2, 4), (4, 8), (8, 12), (12, 16)]
HS_COLS = 2 * KD * P          # SBUF columns per hs block (2048)

_BUILD_CACHE = {}


def _balanced_chunks(total, step):
    """Split `total` into <=step chunks, as equal as possible (32-aligned)."""
    n = -(-total // step)
    base = -(-total // (n * 32)) * 32
    out = []
    o = 0
    while o < total:
        sz = min(base, total - o)
        out.append((o, sz))
        o += sz
    return out


def _build(C, mm_mode):
    """Build the per-core SPMD Bass kernel for token capacity C."""
    n_chunks = _balanced_chunks(C, 512)   # token tiles in the free dim

    mdt = BF16 if mm_mode == "bf16" else F32

    nc = bacc.Bacc("TRN2", target_bir_lowering=False, debug=False)
    # x^T, chunk-blocked then k-blocked: chunk ni at cols KD*n0, inside it
    # k-block k at cols [k*nn:(k+1)*nn] (contiguous per partition).
    xt = nc.dram_tensor("xt", [P, KD * C], mdt, kind="ExternalInput")
    # gate/up packed hs-major: [hs][gate k0..k7 | up k0..k7], 128 cols each
    gu = nc.dram_tensor("gu", [P, 2 * KD * H], mdt, kind="ExternalInput")
    # down-proj k-blocked: block k = dw[k*128:(k+1)*128, :] at cols [k*D:...]
    dw = nc.dram_tensor("dw", [P, KH * D], mdt, kind="ExternalInput")
    # transposed output y^T
    yt = nc.dram_tensor("yt", [D, C], F32, kind="ExternalOutput")

    with tile.TileContext(nc) as tc:
        with (
            tc.tile_pool(name="xp", bufs=2) as xp,
            tc.tile_pool(name="wg", bufs=1) as wg,
            tc.tile_pool(name="w2", bufs=1) as w2,
            tc.tile_pool(name="hp", bufs=KH) as hp,
            tc.tile_pool(name="outp", bufs=4) as outp,
            tc.tile_pool(name="ps", bufs=8, space="PSUM") as ps,
        ):
            dma_engines = [nc.sync, nc.scalar, nc.gpsimd]
            _ld_i = [0]

            def load3(dst, src, cols):
                """One logical load, column-split 3 ways so all three DMA
                queues advance the consumption prefix in lockstep.  The
                queue offset rotates per call so consecutive loads' first
                thirds land on different queues (faster arming)."""
                b = -(-cols // (3 * 32)) * 32
                o = 0
                q0 = _ld_i[0]
                _ld_i[0] += 1
                for q in range(3):
                    sz = min(b, cols - o)
                    if sz <= 0:
                        break
                    dma_engines[(q0 + q) % 3].dma_start(
                        dst[:, o:o + sz], src[:, o:o + sz])
                    o += sz

            # resident tiles
            x_t = [xp.tile([P, KD * nn], mdt, tag="x", name=f"x{i}")
                   for i, (n0, nn) in enumerate(n_chunks)]
            wgt = wg.tile([P, 2 * KD * H], mdt, tag="w", name="wgt")
            dwt = w2.tile([P, KH * D], mdt, tag="dw", name="dwt")
            h_t = [hp.tile([P, C], mdt, tag="h", name=f"h{k}")
                   for k in range(KH)]

            # ---- loads, in strict consumption-priority order ----
            def load_x(ni):
                n0, nn = n_chunks[ni]
                load3(x_t[ni][:, :], xt[:, KD * n0:KD * (n0 + nn)], KD * nn)

            def load_w_group(gi):
                h0, h1 = W_GROUPS[gi]
                c0, c1 = h0 * HS_COLS, h1 * HS_COLS
                load3(wgt[:, c0:c1], gu[:, c0:c1], c1 - c0)

            load_x(0)
            load_w_group(0)
            for ni in range(1, len(n_chunks)):
                load_x(ni)
            for gi in range(1, len(W_GROUPS)):
                load_w_group(gi)
            load3(dwt[:, :], dw[:, :], KH * D)

            def w_slice(k, which, hs):
                off = hs * HS_COLS + which * KD * P + k * P
                return wgt[:, off:off + P]

            def dw_slice(k, dt):
                return dwt[:, k * D + dt * P:k * D + dt * P + P]

            def x_slice(k, ni):
                nn = n_chunks[ni][1]
                return x_t[ni][:, k * nn:(k + 1) * nn]

            def mm1_hs(hs):
                # Both token chunks back-to-back under ONE weight block:
                # each 0.525MB hs block feeds 2x the matmul work, halving
                # the weight-stream bandwidth demand (the HBM is shared by
                # all 8 cores and is the roofline during the load phase).
                ht = h_t[hs]
                for ni, (n0, nn) in enumerate(n_chunks):
                    pa = ps.tile([P, 512], F32, tag="ps", name="pa")[:, :nn]
                    pu = ps.tile([P, 512], F32, tag="ps", name="pu")[:, :nn]
                    # interleave the gate/up accumulation chains: back-to-
                    # back accumulates into one PSUM bank stall the PE
                    for k in range(KD):
                        nc.tensor.matmul(
                            pa[:, :], lhsT=w_slice(k, 0, hs),
                            rhs=x_slice(k, ni),
                            start=(k == 0), stop=(k == KD - 1),
                        )
                        nc.tensor.matmul(
                            pu[:, :], lhsT=w_slice(k, 1, hs),
                            rhs=x_slice(k, ni),
                            start=(k == 0), stop=(k == KD - 1),
                        )
                    nc.scalar.activation(
                        ht[:, n0:n0 + nn], pa[:, :],
                        mybir.ActivationFunctionType.Silu,
                    )
                    nc.vector.tensor_mul(
                        ht[:, n0:n0 + nn], ht[:, n0:n0 + nn], pu[:, :]
                    )

            def mm2(ni):
                n0, nn = n_chunks[ni]
                for dt in range(ND):
                    py = ps.tile([P, 512], F32, tag="ps", name="py")[:, :nn]
                    for k in range(KH):
                        nc.tensor.matmul(
                            py[:, :], lhsT=dw_slice(k, dt),
                            rhs=h_t[k][:, n0:n0 + nn],
                            start=(k == 0), stop=(k == KH - 1),
                        )
                    ot = outp.tile([P, 512], F32, tag="out", name="ot")[:, :nn]
                    nc.vector.tensor_copy(ot[:, :], py[:, :])
                    dma_engines[dt % 2].dma_start(
                        yt[dt * P:(dt + 1) * P, n0:n0 + nn], ot[:, :])

            # mm2 strictly after all mm1 so gu loads never compete with dw;
            # outputs drain over mm2's ~25us instead of one tail burst.
            for hs in range(KH):
                mm1_hs(hs)
            for ni in range(len(n_chunks)):
                mm2(ni)

    nc.compile()
    return nc


def _get_kernel(C, mm_mode=None):
    """Build (cached).  Falls back to exact fp32 if the bf16 build fails."""
    mm_mode = mm_mode or MM_MODE
    key = (C, mm_mode)
    if key not in _BUILD_CACHE:
        try:
            _BUILD_CACHE[key] = (_build(C, mm_mode), mm_mode)
        except Exception:
            if mm_mode == "fp32":
                raise
            _BUILD_CACHE[key] = (_build(C, "fp32"), "fp32")
    return _BUILD_CACHE[key]


def _route(xf, gate_w):
    """argmax expert per token, computed in fp64 on host (negligible work)."""
    logits = xf.astype(np.float64) @ np.asarray(gate_w, np.float64).T
    return logits.argmax(axis=1)


def _pack_gu(gw_e, uw_e, ndt):
    """[128, 2*KD*H] hs-major: [hs][gate k0..k7 | up k0..k7] x 128 cols."""
    g = gw_e.reshape(KD, P, KH, P).transpose(1, 2, 0, 3)   # [P, hs, k, 128]
    u = uw_e.reshape(KD, P, KH, P).transpose(1, 2, 0, 3)
    return np.ascontiguousarray(
        np.stack([g, u], axis=2).reshape(P, 2 * KD * H)).astype(ndt)


def _pack_dw(dw_e, ndt):
    """[128, KH*D]: k-blocked partition-major."""
    return np.ascontiguousarray(
        dw_e.reshape(KH, P, D).transpose(1, 0, 2).reshape(P, KH * D)
    ).astype(ndt)


def _pack_x(xe, chunks):
    """[128, KD*C]: chunk-blocked, k-blocked inside each chunk."""
    parts = []
    for n0, nn in chunks:
        parts.append(
            xe[:, n0:n0 + nn].reshape(KD, P, nn).transpose(1, 0, 2)
            .reshape(P, KD * nn))
    return np.ascontiguousarray(np.concatenate(parts, axis=1))


def kernel(x, gate_w, gate_bank, up_bank, down_bank):
    x = np.asarray(x, np.float32)
    assert x.shape == (B, T, D)

    xf = np.ascontiguousarray(x.reshape(BT, D))
    sel = _route(xf, gate_w)
    idx = [np.nonzero(sel == e)[0] for e in range(E)]
    maxc = max(len(i) for i in idx)
    C = max(P, -(-maxc // 32) * 32)
    chunks = _balanced_chunks(C, 512)

    nc, mode = _get_kernel(C)

    ndt = BF16_NP if mode == "bf16" else np.float32
    gate_bank = np.asarray(gate_bank, np.float32)
    up_bank = np.asarray(up_bank, np.float32)
    down_bank = np.asarray(down_bank, np.float32)

    in_maps = []
    for e in range(E):
        xe = np.zeros((D, C), ndt)
        n = len(idx[e])
        if n:
            xe[:, :n] = xf[idx[e]].T.astype(ndt)
        in_maps.append({
            "xt": _pack_x(xe, chunks),
            "gu": _pack_gu(gate_bank[e], up_bank[e], ndt),
            "dw": _pack_dw(down_bank[e], ndt),
        })

    res = run_bass_kernel_spmd(nc, in_maps, core_ids=list(range(NCORES)))

    out = np.empty((BT, D), np.float32)
    for e in range(E):
        n = len(idx[e])
        if n:
            out[idx[e]] = res.results[e]["yt"][:, :n].T
    return out.reshape(B, T, D)
